# revision 2
# baseline (speedup 1.0000x reference)
"""Trainium2 Bass kernel for the FCBlock weight-transform + matmul problem.

Math (per reference):
    W_i = per-head 3x3 conv over W.reshape(4, 1024, 4096) + conv_b
          + sigmoid(sk_wt) * W            (per-head scalars)
    out  = inp @ W_i.T                    (inp: [2, 2048, 4096])

Strategy: tensor-parallel shard of W_i along fout across 8 cores
(512 rows each, exactly half a head per core).  The host pre-computes
layout/constant data only:
  - xs  = inp^T cast to fp8e4 [fin, tok]  (replicated to all cores),
  - wth = W^T shard [fin, 514] bf16 with a 1-col fout-halo, zero-padded
    at head boundaries,
  - hh  = per-fin-window conv halo rows (pre-gathered, pre-shifted),
  - b3/hm = tiny banded/halo coefficient matrices from conv_w + sk_wt,
  - rsb = conv_b[h] * rowsum(inp) broadcast [128, tok] f32 (the rank-1
    bias term of the matmul, applied exactly in f32 on device).
On each core:
  - run the 3x3-conv weight transform directly in the TRANSPOSED
    orientation (fin on partitions) as banded PE matmuls, so W_i^T
    needs no PE transposes; drain PSUM straight to fp8 with a x128
    scale (conv bias EXCLUDED -- it would eat the fp8 mantissa),
  - main matmul in fp8 DoubleRow mode (K virtualized to 256):
    out^T[f, t] = sum_c W_i^T[c, f] * x^T[c, t], f32 PSUM accumulation;
    drain applies the 1/128 scale and adds the exact rank-1 bias term.
Output is out^T sharded on fout; the host concatenates + transposes.
Measured absmax rel err ~1.3e-2 (vs 2e-2 gate).
"""

import numpy as np

import concourse.bass as bass
import concourse.mybir as mybir
import concourse.tile as tile
from concourse import bacc
from concourse.bass_utils import run_bass_kernel_spmd

F32 = mybir.dt.float32
BF16 = mybir.dt.bfloat16
FP8 = mybir.dt.float8e4

NCORES = 8
NUM_HEADS = 4
TOK = 4096          # 2 * 2048 tokens
FIN = 4096
FOUT = 4096
FSH = FOUT // NCORES  # 512 fout rows per core
WSCALE = 128.0       # fp8 weight scale (power of two)

AF = mybir.ActivationFunctionType
ALU = mybir.AluOpType


def _ap(t):
    return t.tensor if hasattr(t, "tensor") else t


def build_program(tok=TOK, fin=FIN):
    """Build the per-core SPMD program (tok parameterized for sim)."""
    assert tok % 512 == 0 and fin % 512 == 0
    n_k = fin // 128             # fin 128-blocks (contraction)
    n_g = n_k // 2               # DoubleRow 256-deep groups
    n_tc = tok // 512            # 512-token output chunks
    n_f = FSH // 128             # fout 128-blocks per core
    n_wc = n_k // 4              # 4-window transform DMA chunks

    nc = bacc.Bacc(None, target_bir_lowering=False)

    xs = nc.declare_dram_parameter("xs", [fin, tok], FP8, isOutput=False)
    wth = nc.declare_dram_parameter("wth", [fin, FSH + 2], BF16, isOutput=False)
    hh = nc.declare_dram_parameter("hh", [n_k, 6, FSH], BF16, isOutput=False)
    b3 = nc.declare_dram_parameter("b3", [128, 3, 128], BF16, isOutput=False)
    hmd = nc.declare_dram_parameter("hm", [6, 128], BF16, isOutput=False)
    rsb = nc.declare_dram_parameter("rsb", [128, tok], F32, isOutput=False)
    out = nc.declare_dram_parameter("o", [FSH, tok], F32, isOutput=True)

    with tile.TileContext(nc) as tc:
        with (
            tc.tile_pool(name="const", bufs=1) as const,
            tc.tile_pool(name="wtpool", bufs=1) as wtpool,
            tc.tile_pool(name="wfp", bufs=3) as wfp,
            tc.tile_pool(name="xb", bufs=3) as xbp,
            tc.tile_pool(name="osb", bufs=4) as osbp,
            tc.tile_pool(name="psw", bufs=3, space="PSUM") as psw,
            tc.tile_pool(name="psm", bufs=5, space="PSUM") as psm,
        ):
            # ---- HAM warmup: dummy matmuls while the first DMAs land -------
            warm = const.tile([128, 512], BF16)
            nc.vector.memset(warm[:], 1.0)
            for i in range(14):
                pwu = psw.tile([128, 512], F32, tag="pw")
                nc.tensor.matmul(pwu[:], warm[:, :128], warm[:],
                                 start=True, stop=True)

            # ---- constants (host-built), transform inputs ------------------
            # throwaway DMA: absorbs the DMA-engine cold-start before the
            # latency-critical b3/wth loads (result unused)
            dume = const.tile([128, 256], BF16)
            nc.scalar.dma_start(
                out=dume[:],
                in_=bass.AP(_ap(wth), 0, [[FSH + 2, 128], [1, 256]]))
            b3_sb = const.tile([128, 3, 128], BF16)
            nc.scalar.dma_start(out=b3_sb[:], in_=b3[:])
            hm = const.tile([6, 128], BF16)
            hf_all = const.tile([6, n_k, FSH], BF16)
            rs_sb = const.tile([128, tok], F32)

            # W_i^T (fp8, x128), fin on partitions: [128, n_k, FSH]
            wt8 = wtpool.tile([128, n_k, FSH], FP8)

            # ---- phase T: weight transform (transposed orientation) --------
            for wc in range(n_wc):
                wf4 = wfp.tile([128, 4, FSH + 2], BF16, tag="wf")
                nc.scalar.dma_start(
                    out=wf4[:],
                    in_=bass.AP(_ap(wth), 512 * wc * (FSH + 2),
                                [[FSH + 2, 128], [128 * (FSH + 2), 4],
                                 [1, FSH + 2]]))
                if wc == 0:
                    # halo inputs after the first band-data chunk: needed
                    # by window 0's 4th matmul, not before
                    nc.scalar.dma_start(out=hm[:], in_=hmd[:])
                    nc.scalar.dma_start(
                        out=hf_all[:],
                        in_=bass.AP(_ap(hh), 0,
                                    [[FSH, 6], [6 * FSH, n_k], [1, FSH]]))
                for ki in range(4):
                    k = 4 * wc + ki
                    pw = psw.tile([128, FSH], F32, tag="pw")
                    for dr in range(3):
                        nc.tensor.matmul(pw[:], b3_sb[:, dr, :],
                                         wf4[:, ki, dr:dr + FSH],
                                         start=(dr == 0), stop=False)
                    nc.tensor.matmul(pw[:], hm[:], hf_all[:, k, :],
                                     start=False, stop=True)
                    if k % 2 == 0:
                        nc.scalar.activation(wt8[:, k, :], pw[:], AF.Copy,
                                             scale=WSCALE)
                    else:
                        nc.vector.tensor_scalar(wt8[:, k, :], pw[:], WSCALE,
                                                None, ALU.mult)

            # rank-1 term: after the wth chunks on the scalar queue
            nc.scalar.dma_start(out=rs_sb[:], in_=rsb[:])

            # ---- phase M: main matmul (fp8 DoubleRow), out^T form ----------
            for t in range(n_tc):
                xt = xbp.tile([128, n_k, 512], FP8, tag="xt")
                nc.scalar.dma_start(
                    out=xt[:],
                    in_=bass.AP(_ap(xs), 512 * t,
                                [[tok, 128], [128 * tok, n_k], [1, 512]]))
                for f in range(n_f):
                    po = psm.tile([128, 512], F32, tag="po")
                    for g in range(n_g):
                        nc.tensor.matmul(
                            po[:],
                            wt8[:, 2 * g:2 * g + 2, 128 * f:128 * f + 128],
                            xt[:, 2 * g:2 * g + 2, :],
                            start=(g == 0), stop=(g == n_g - 1),
                            perf_mode=mybir.MatmulPerfMode.DoubleRow)
                    ob = osbp.tile([128, 512], F32, tag="ob")
                    nc.vector.scalar_tensor_tensor(
                        out=ob[:], in0=po[:], scalar=1.0 / WSCALE,
                        in1=rs_sb[:, 512 * t:512 * t + 512],
                        op0=ALU.mult, op1=ALU.add)
                    nc.gpsimd.dma_start(
                        out=out[128 * f:128 * f + 128, 512 * t:512 * t + 512],
                        in_=ob[:])

    nc.compile()
    return nc


def shard_inputs(inp, W, conv_w, conv_b, sk_wt, fin=FIN):
    """Build the 8 per-core input maps (host: layout/constants only)."""
    tok = inp.size // fin
    f8 = mybir.dt.np(FP8)
    bf = mybir.dt.np(BF16)
    n_k = fin // 128

    inp2 = np.asarray(inp, dtype=np.float32).reshape(tok, fin)
    W = np.asarray(W, dtype=np.float32)
    cw = np.asarray(conv_w, dtype=np.float32).reshape(NUM_HEADS, 3, 3)
    cb = np.asarray(conv_b, dtype=np.float32).reshape(NUM_HEADS)
    sk = np.asarray(sk_wt, dtype=np.float32).reshape(NUM_HEADS)
    sig = 1.0 / (1.0 + np.exp(-sk))

    # x^T in fp8 (replicated), rank-1 rowsum term in f32
    xT8 = np.ascontiguousarray(inp2.T).astype(f8)
    rs = inp2.astype(np.float64).sum(axis=1).astype(np.float32)  # [tok]

    hsz = W.shape[0] // NUM_HEADS
    in_maps = []
    for c in range(NCORES):
        gr0 = c * FSH
        h = (gr0 // hsz) % NUM_HEADS
        lo = max(gr0 - 1, h * hsz)          # fout halo, head-clamped
        hi = min(gr0 + FSH + 1, (h + 1) * hsz)
        # W^T shard with fout-halo: wth[c_, j] = W[gr0 + j - 1, c_]
        wthal = np.zeros((fin, FSH + 2), dtype=np.float32)
        wthal[:, lo - (gr0 - 1):hi - (gr0 - 1)] = W[lo:hi, :fin].T
        # conv halo rows: hh[k, 3r+dr, j] = W[gr0+j+dr-1, c_r], head-clamped
        hhv = np.zeros((n_k, 6, FSH), dtype=np.float32)
        for k in range(n_k):
            for r, c_r in ((0, 128 * k - 1), (1, 128 * k + 128)):
                if c_r < 0 or c_r >= fin:
                    continue
                col = W[lo:hi, c_r]
                for dr in range(3):
                    j0 = lo - (gr0 + dr - 1)
                    jlo = max(0, j0)
                    jhi = min(FSH, hi - (gr0 + dr - 1))
                    if jhi > jlo:
                        hhv[k, 3 * r + dr, jlo:jhi] = col[jlo - j0:jhi - j0]
        # banded matrices B_dr[c_in, c_out] = cw[dr, dc] where
        # c_in - c_out == dc - 1; sigmoid residual on B_1's main diagonal
        b3v = np.zeros((128, 3, 128), dtype=np.float32)
        idx = np.arange(127)
        for dr in range(3):
            b3v[idx, dr, idx + 1] = cw[h, dr, 0]
            b3v[np.arange(128), dr, np.arange(128)] = cw[h, dr, 1]
            b3v[idx + 1, dr, idx] = cw[h, dr, 2]
        b3v[np.arange(128), 1, np.arange(128)] += sig[h]
        # halo coefficients: row (r=0,dr) -> c_out 0 tap cw[dr,0];
        #                    row (r=1,dr) -> c_out 127 tap cw[dr,2]
        hmv = np.zeros((6, 128), dtype=np.float32)
        for dr in range(3):
            hmv[dr, 0] = cw[h, dr, 0]
            hmv[3 + dr, 127] = cw[h, dr, 2]
        rsbv = np.ascontiguousarray(
            np.broadcast_to(cb[h] * rs, (128, tok))).astype(np.float32)
        in_maps.append({
            "xs": xT8,
            "wth": wthal.astype(bf),
            "hh": hhv.astype(bf),
            "b3": b3v.astype(bf),
            "hm": hmv.astype(bf),
            "rsb": rsbv,
        })
    return in_maps


_PROGRAM_CACHE = {}


def _get_program(tok, fin):
    key = (tok, fin)
    if key not in _PROGRAM_CACHE:
        _PROGRAM_CACHE[key] = build_program(tok, fin)
    return _PROGRAM_CACHE[key]


def kernel(inp, W, conv_w, conv_b, sk_wt):
    nc = _get_program(TOK, FIN)
    in_maps = shard_inputs(inp, W, conv_w, conv_b, sk_wt)
    res = run_bass_kernel_spmd(nc, in_maps, list(range(NCORES)))
    # out^T shards [FSH, tok] -> [fout, tok] -> transpose to [tok, fout]
    oT = np.concatenate([res.results[c]["o"] for c in range(NCORES)], axis=0)
    return np.ascontiguousarray(oT.T.reshape(2, TOK // 2, FOUT)
                                ).astype(np.float32)


# revision 3
# speedup vs baseline: 1.0087x; 1.0087x over previous
"""Trainium2 Bass kernel for the FCBlock weight-transform + matmul problem.

Math (per reference):
    W_i = per-head 3x3 conv over W.reshape(4, 1024, 4096) + conv_b
          + sigmoid(sk_wt) * W            (per-head scalars)
    out  = inp @ W_i.T                    (inp: [2, 2048, 4096])

Strategy: tensor-parallel shard of W_i along fout across 8 cores
(512 rows each, exactly half a head per core).  The host pre-computes
layout/constant data only:
  - xs  = inp^T cast to fp8e4 [fin, tok]  (replicated to all cores),
  - wth = W^T shard [fin, 514] bf16 with a 1-col fout-halo, zero-padded
    at head boundaries,
  - hh  = per-fin-window conv halo rows (pre-gathered, pre-shifted),
  - b3/hm = tiny banded/halo coefficient matrices from conv_w + sk_wt,
  - rsb = conv_b[h] * rowsum(inp) broadcast [128, tok] f32 (the rank-1
    bias term of the matmul, applied exactly in f32 on device).
On each core:
  - run the 3x3-conv weight transform directly in the TRANSPOSED
    orientation (fin on partitions) as banded PE matmuls, so W_i^T
    needs no PE transposes; drain PSUM straight to fp8 with a x128
    scale (conv bias EXCLUDED -- it would eat the fp8 mantissa),
  - main matmul in fp8 DoubleRow mode (K virtualized to 256):
    out^T[f, t] = sum_c W_i^T[c, f] * x^T[c, t], f32 PSUM accumulation;
    drain applies the 1/128 scale and adds the exact rank-1 bias term.
Output is out^T sharded on fout; the host concatenates + transposes.
Measured absmax rel err ~1.3e-2 (vs 2e-2 gate).
"""

import numpy as np

import concourse.bass as bass
import concourse.mybir as mybir
import concourse.tile as tile
from concourse import bacc
from concourse.bass_utils import run_bass_kernel_spmd

F32 = mybir.dt.float32
BF16 = mybir.dt.bfloat16
FP8 = mybir.dt.float8e4

NCORES = 8
NUM_HEADS = 4
TOK = 4096          # 2 * 2048 tokens
FIN = 4096
FOUT = 4096
FSH = FOUT // NCORES  # 512 fout rows per core
WSCALE = 128.0       # fp8 weight scale (power of two)

AF = mybir.ActivationFunctionType
ALU = mybir.AluOpType


def _ap(t):
    return t.tensor if hasattr(t, "tensor") else t


def build_program(tok=TOK, fin=FIN):
    """Build the per-core SPMD program (tok parameterized for sim)."""
    assert tok % 512 == 0 and fin % 512 == 0
    n_k = fin // 128             # fin 128-blocks (contraction)
    n_g = n_k // 2               # DoubleRow 256-deep groups
    n_tc = tok // 512            # 512-token output chunks
    n_f = FSH // 128             # fout 128-blocks per core
    n_wc = n_k // 4              # 4-window transform DMA chunks

    nc = bacc.Bacc(None, target_bir_lowering=False)

    xs = nc.declare_dram_parameter("xs", [fin, tok], FP8, isOutput=False)
    wth = nc.declare_dram_parameter("wth", [fin, FSH + 2], BF16, isOutput=False)
    hh = nc.declare_dram_parameter("hh", [n_k, 6, FSH], BF16, isOutput=False)
    b3 = nc.declare_dram_parameter("b3", [128, 3, 128], BF16, isOutput=False)
    hmd = nc.declare_dram_parameter("hm", [6, 128], BF16, isOutput=False)
    rsb = nc.declare_dram_parameter("rsb", [128, tok], F32, isOutput=False)
    out = nc.declare_dram_parameter("o", [FSH, tok], F32, isOutput=True)

    with tile.TileContext(nc) as tc:
        with (
            tc.tile_pool(name="const", bufs=1) as const,
            tc.tile_pool(name="wtpool", bufs=1) as wtpool,
            tc.tile_pool(name="wfp", bufs=3) as wfp,
            tc.tile_pool(name="xb", bufs=3) as xbp,
            tc.tile_pool(name="osb", bufs=4) as osbp,
            tc.tile_pool(name="psw", bufs=3, space="PSUM") as psw,
            tc.tile_pool(name="psm", bufs=5, space="PSUM") as psm,
        ):
            # ---- HAM warmup: dummy matmuls while the first DMAs land -------
            warm = const.tile([128, 512], BF16)
            nc.vector.memset(warm[:], 1.0)
            for i in range(20):
                pwu = psw.tile([128, 512], F32, tag="pw")
                nc.tensor.matmul(pwu[:], warm[:, :128], warm[:],
                                 start=True, stop=True)

            # ---- constants (host-built), transform inputs ------------------
            # throwaway DMA: absorbs the DMA-engine cold-start before the
            # latency-critical b3/wth loads (result unused)
            dume = const.tile([128, 256], BF16)
            nc.scalar.dma_start(
                out=dume[:],
                in_=bass.AP(_ap(wth), 0, [[FSH + 2, 128], [1, 256]]))
            b3_sb = const.tile([128, 3, 128], BF16)
            nc.scalar.dma_start(out=b3_sb[:], in_=b3[:])
            hm = const.tile([6, 128], BF16)
            hf_all = const.tile([6, n_k, FSH], BF16)
            rs_sb = const.tile([128, tok], F32)

            # W_i^T (fp8, x128), fin on partitions: [128, n_k, FSH]
            wt8 = wtpool.tile([128, n_k, FSH], FP8)

            # ---- phase T: weight transform (transposed orientation) --------
            for wc in range(n_wc):
                wf4 = wfp.tile([128, 4, FSH + 2], BF16, tag="wf")
                nc.scalar.dma_start(
                    out=wf4[:],
                    in_=bass.AP(_ap(wth), 512 * wc * (FSH + 2),
                                [[FSH + 2, 128], [128 * (FSH + 2), 4],
                                 [1, FSH + 2]]))
                if wc == 0:
                    # halo inputs after the first band-data chunk: needed
                    # by window 0's 4th matmul, not before
                    nc.scalar.dma_start(out=hm[:], in_=hmd[:])
                    nc.scalar.dma_start(
                        out=hf_all[:],
                        in_=bass.AP(_ap(hh), 0,
                                    [[FSH, 6], [6 * FSH, n_k], [1, FSH]]))
                for ki in range(4):
                    k = 4 * wc + ki
                    pw = psw.tile([128, FSH], F32, tag="pw")
                    for dr in range(3):
                        nc.tensor.matmul(pw[:], b3_sb[:, dr, :],
                                         wf4[:, ki, dr:dr + FSH],
                                         start=(dr == 0), stop=False)
                    nc.tensor.matmul(pw[:], hm[:], hf_all[:, k, :],
                                     start=False, stop=True)
                    if k % 2 == 0:
                        nc.scalar.activation(wt8[:, k, :], pw[:], AF.Copy,
                                             scale=WSCALE)
                    else:
                        nc.vector.tensor_scalar(wt8[:, k, :], pw[:], WSCALE,
                                                None, ALU.mult)

            # rank-1 term: after the wth chunks on the scalar queue
            nc.scalar.dma_start(out=rs_sb[:], in_=rsb[:])

            # ---- phase M: main matmul (fp8 DoubleRow), out^T form ----------
            for t in range(n_tc):
                xt = xbp.tile([128, n_k, 512], FP8, tag="xt")
                nc.scalar.dma_start(
                    out=xt[:],
                    in_=bass.AP(_ap(xs), 512 * t,
                                [[tok, 128], [128 * tok, n_k], [1, 512]]))
                for f in range(n_f):
                    po = psm.tile([128, 512], F32, tag="po")
                    for g in range(n_g):
                        nc.tensor.matmul(
                            po[:],
                            wt8[:, 2 * g:2 * g + 2, 128 * f:128 * f + 128],
                            xt[:, 2 * g:2 * g + 2, :],
                            start=(g == 0), stop=(g == n_g - 1),
                            perf_mode=mybir.MatmulPerfMode.DoubleRow)
                    ob = osbp.tile([128, 512], F32, tag="ob")
                    nc.vector.scalar_tensor_tensor(
                        out=ob[:], in0=po[:], scalar=1.0 / WSCALE,
                        in1=rs_sb[:, 512 * t:512 * t + 512],
                        op0=ALU.mult, op1=ALU.add)
                    nc.sync.dma_start(
                        out=out[128 * f:128 * f + 128, 512 * t:512 * t + 512],
                        in_=ob[:])

    nc.compile()
    return nc


def shard_inputs(inp, W, conv_w, conv_b, sk_wt, fin=FIN):
    """Build the 8 per-core input maps (host: layout/constants only)."""
    tok = inp.size // fin
    f8 = mybir.dt.np(FP8)
    bf = mybir.dt.np(BF16)
    n_k = fin // 128

    inp2 = np.asarray(inp, dtype=np.float32).reshape(tok, fin)
    W = np.asarray(W, dtype=np.float32)
    cw = np.asarray(conv_w, dtype=np.float32).reshape(NUM_HEADS, 3, 3)
    cb = np.asarray(conv_b, dtype=np.float32).reshape(NUM_HEADS)
    sk = np.asarray(sk_wt, dtype=np.float32).reshape(NUM_HEADS)
    sig = 1.0 / (1.0 + np.exp(-sk))

    # x^T in fp8 (replicated), rank-1 rowsum term in f32
    xT8 = np.ascontiguousarray(inp2.T).astype(f8)
    rs = inp2.astype(np.float64).sum(axis=1).astype(np.float32)  # [tok]

    hsz = W.shape[0] // NUM_HEADS
    in_maps = []
    for c in range(NCORES):
        gr0 = c * FSH
        h = (gr0 // hsz) % NUM_HEADS
        lo = max(gr0 - 1, h * hsz)          # fout halo, head-clamped
        hi = min(gr0 + FSH + 1, (h + 1) * hsz)
        # W^T shard with fout-halo: wth[c_, j] = W[gr0 + j - 1, c_]
        wthal = np.zeros((fin, FSH + 2), dtype=np.float32)
        wthal[:, lo - (gr0 - 1):hi - (gr0 - 1)] = W[lo:hi, :fin].T
        # conv halo rows: hh[k, 3r+dr, j] = W[gr0+j+dr-1, c_r], head-clamped
        hhv = np.zeros((n_k, 6, FSH), dtype=np.float32)
        for k in range(n_k):
            for r, c_r in ((0, 128 * k - 1), (1, 128 * k + 128)):
                if c_r < 0 or c_r >= fin:
                    continue
                col = W[lo:hi, c_r]
                for dr in range(3):
                    j0 = lo - (gr0 + dr - 1)
                    jlo = max(0, j0)
                    jhi = min(FSH, hi - (gr0 + dr - 1))
                    if jhi > jlo:
                        hhv[k, 3 * r + dr, jlo:jhi] = col[jlo - j0:jhi - j0]
        # banded matrices B_dr[c_in, c_out] = cw[dr, dc] where
        # c_in - c_out == dc - 1; sigmoid residual on B_1's main diagonal
        b3v = np.zeros((128, 3, 128), dtype=np.float32)
        idx = np.arange(127)
        for dr in range(3):
            b3v[idx, dr, idx + 1] = cw[h, dr, 0]
            b3v[np.arange(128), dr, np.arange(128)] = cw[h, dr, 1]
            b3v[idx + 1, dr, idx] = cw[h, dr, 2]
        b3v[np.arange(128), 1, np.arange(128)] += sig[h]
        # halo coefficients: row (r=0,dr) -> c_out 0 tap cw[dr,0];
        #                    row (r=1,dr) -> c_out 127 tap cw[dr,2]
        hmv = np.zeros((6, 128), dtype=np.float32)
        for dr in range(3):
            hmv[dr, 0] = cw[h, dr, 0]
            hmv[3 + dr, 127] = cw[h, dr, 2]
        rsbv = np.ascontiguousarray(
            np.broadcast_to(cb[h] * rs, (128, tok))).astype(np.float32)
        in_maps.append({
            "xs": xT8,
            "wth": wthal.astype(bf),
            "hh": hhv.astype(bf),
            "b3": b3v.astype(bf),
            "hm": hmv.astype(bf),
            "rsb": rsbv,
        })
    return in_maps


_PROGRAM_CACHE = {}


def _get_program(tok, fin):
    key = (tok, fin)
    if key not in _PROGRAM_CACHE:
        _PROGRAM_CACHE[key] = build_program(tok, fin)
    return _PROGRAM_CACHE[key]


def kernel(inp, W, conv_w, conv_b, sk_wt):
    nc = _get_program(TOK, FIN)
    in_maps = shard_inputs(inp, W, conv_w, conv_b, sk_wt)
    res = run_bass_kernel_spmd(nc, in_maps, list(range(NCORES)))
    # out^T shards [FSH, tok] -> [fout, tok] -> transpose to [tok, fout]
    oT = np.concatenate([res.results[c]["o"] for c in range(NCORES)], axis=0)
    return np.ascontiguousarray(oT.T.reshape(2, TOK // 2, FOUT)
                                ).astype(np.float32)


# revision 4
# speedup vs baseline: 1.0353x; 1.0263x over previous
"""Trainium2 Bass kernel for the FCBlock weight-transform + matmul problem.

Math (per reference):
    W_i = per-head 3x3 conv over W.reshape(4, 1024, 4096) + conv_b
          + sigmoid(sk_wt) * W            (per-head scalars)
    out  = inp @ W_i.T                    (inp: [2, 2048, 4096])

Strategy: tensor-parallel shard of W_i along fout across 8 cores
(512 rows each, exactly half a head per core).  The host pre-computes
layout/constant data only:
  - xs  = inp^T cast to fp8e4 [fin, tok]  (replicated to all cores),
  - wth = W^T shard [fin, 514] bf16 with a 1-col fout-halo, zero-padded
    at head boundaries,
  - hh  = per-fin-window conv halo rows (pre-gathered, pre-shifted),
  - b3/hm = tiny banded/halo coefficient matrices from conv_w + sk_wt,
  - rsb = conv_b[h] * rowsum(inp) broadcast [128, tok] f32 (the rank-1
    bias term of the matmul, applied exactly in f32 on device).
On each core:
  - run the 3x3-conv weight transform directly in the TRANSPOSED
    orientation (fin on partitions) as banded PE matmuls, so W_i^T
    needs no PE transposes; drain PSUM straight to fp8 with a x128
    scale (conv bias EXCLUDED -- it would eat the fp8 mantissa),
  - main matmul in fp8 DoubleRow mode (K virtualized to 256):
    out^T[f, t] = sum_c W_i^T[c, f] * x^T[c, t], f32 PSUM accumulation;
    drain applies the 1/128 scale and adds the exact rank-1 bias term.
Output is out^T sharded on fout; the host concatenates + transposes.
Measured absmax rel err ~1.3e-2 (vs 2e-2 gate).
"""

import numpy as np

import concourse.bass as bass
import concourse.mybir as mybir
import concourse.tile as tile
from concourse import bacc
from concourse.bass_utils import run_bass_kernel_spmd

F32 = mybir.dt.float32
BF16 = mybir.dt.bfloat16
FP8 = mybir.dt.float8e4

NCORES = 8
NUM_HEADS = 4
TOK = 4096          # 2 * 2048 tokens
FIN = 4096
FOUT = 4096
FSH = FOUT // NCORES  # 512 fout rows per core
WSCALE = 128.0       # fp8 weight scale (power of two)

AF = mybir.ActivationFunctionType
ALU = mybir.AluOpType


def _ap(t):
    return t.tensor if hasattr(t, "tensor") else t


def build_program(tok=TOK, fin=FIN):
    """Build the per-core SPMD program (tok parameterized for sim)."""
    assert tok % 512 == 0 and fin % 512 == 0
    n_k = fin // 128             # fin 128-blocks (contraction)
    n_g = n_k // 2               # DoubleRow 256-deep groups
    n_tc = tok // 512            # 512-token output chunks
    n_f = FSH // 128             # fout 128-blocks per core
    n_wc = n_k // 4              # 4-window transform DMA chunks

    nc = bacc.Bacc(None, target_bir_lowering=False)

    xs = nc.declare_dram_parameter("xs", [fin, tok], FP8, isOutput=False)
    wth = nc.declare_dram_parameter("wth", [fin, FSH + 2], BF16, isOutput=False)
    hh = nc.declare_dram_parameter("hh", [n_k, 6, FSH], BF16, isOutput=False)
    b3 = nc.declare_dram_parameter("b3", [128, 3, 128], BF16, isOutput=False)
    hmd = nc.declare_dram_parameter("hm", [6, 128], BF16, isOutput=False)
    rsb = nc.declare_dram_parameter("rsb", [128, tok], F32, isOutput=False)
    out = nc.declare_dram_parameter("o", [FSH, tok], F32, isOutput=True)

    with tile.TileContext(nc) as tc:
        with (
            tc.tile_pool(name="const", bufs=1) as const,
            tc.tile_pool(name="wtpool", bufs=1) as wtpool,
            tc.tile_pool(name="wfp", bufs=3) as wfp,
            tc.tile_pool(name="xb", bufs=3) as xbp,
            tc.tile_pool(name="osb", bufs=4) as osbp,
            tc.tile_pool(name="psw", bufs=4, space="PSUM") as psw,
            tc.tile_pool(name="psm", bufs=4, space="PSUM") as psm,
        ):
            # ---- HAM warmup: dummy matmuls while the first DMAs land -------
            warm = const.tile([128, 512], BF16)
            nc.vector.memset(warm[:], 1.0)
            for i in range(26):
                pwu = psw.tile([128, 512], F32, tag="pw")
                nc.tensor.matmul(pwu[:], warm[:, :128], warm[:],
                                 start=True, stop=True)

            # ---- constants (host-built), transform inputs ------------------
            # throwaway DMA: absorbs the DMA-engine cold-start before the
            # latency-critical b3/wth loads (result unused)
            dume = const.tile([128, 256], BF16)
            nc.scalar.dma_start(
                out=dume[:],
                in_=bass.AP(_ap(wth), 0, [[FSH + 2, 128], [1, 256]]))
            b3_sb = const.tile([128, 3, 128], BF16)
            nc.scalar.dma_start(out=b3_sb[:], in_=b3[:])
            hm = const.tile([6, 128], BF16)
            hf_all = const.tile([6, n_k, FSH], BF16)
            rs_sb = const.tile([128, tok], F32)

            # W_i^T (fp8, x128), fin on partitions: [128, n_k, FSH]
            wt8 = wtpool.tile([128, n_k, FSH], FP8)

            # ---- phase T: weight transform (transposed orientation) --------
            for wc in range(n_wc):
                wf4 = wfp.tile([128, 4, FSH + 2], BF16, tag="wf")
                nc.scalar.dma_start(
                    out=wf4[:],
                    in_=bass.AP(_ap(wth), 512 * wc * (FSH + 2),
                                [[FSH + 2, 128], [128 * (FSH + 2), 4],
                                 [1, FSH + 2]]))
                if wc == 0:
                    # halo inputs after the first band-data chunk: needed
                    # by window 0's 4th matmul, not before
                    nc.scalar.dma_start(out=hm[:], in_=hmd[:])
                    nc.scalar.dma_start(
                        out=hf_all[:],
                        in_=bass.AP(_ap(hh), 0,
                                    [[FSH, 6], [6 * FSH, n_k], [1, FSH]]))
                for ki in range(4):
                    k = 4 * wc + ki
                    pw = psw.tile([128, FSH], F32, tag="pw")
                    for dr in range(3):
                        nc.tensor.matmul(pw[:], b3_sb[:, dr, :],
                                         wf4[:, ki, dr:dr + FSH],
                                         start=(dr == 0), stop=False)
                    nc.tensor.matmul(pw[:], hm[:], hf_all[:, k, :],
                                     start=False, stop=True)
                    if k % 2 == 0:
                        nc.scalar.activation(wt8[:, k, :], pw[:], AF.Copy,
                                             scale=WSCALE)
                    else:
                        nc.vector.tensor_scalar(wt8[:, k, :], pw[:], WSCALE,
                                                None, ALU.mult)

            # rank-1 term: after the wth chunks on the scalar queue
            nc.scalar.dma_start(out=rs_sb[:], in_=rsb[:])

            # ---- phase M: main matmul (fp8 DoubleRow), out^T form ----------
            for t in range(n_tc):
                xt = xbp.tile([128, n_k, 512], FP8, tag="xt")
                nc.scalar.dma_start(
                    out=xt[:],
                    in_=bass.AP(_ap(xs), 512 * t,
                                [[tok, 128], [128 * tok, n_k], [1, 512]]))
                for f in range(n_f):
                    po = psm.tile([128, 512], F32, tag="po")
                    for g in range(n_g):
                        nc.tensor.matmul(
                            po[:],
                            wt8[:, 2 * g:2 * g + 2, 128 * f:128 * f + 128],
                            xt[:, 2 * g:2 * g + 2, :],
                            start=(g == 0), stop=(g == n_g - 1),
                            perf_mode=mybir.MatmulPerfMode.DoubleRow)
                    ob = osbp.tile([128, 512], F32, tag="ob")
                    nc.vector.scalar_tensor_tensor(
                        out=ob[:], in0=po[:], scalar=1.0 / WSCALE,
                        in1=rs_sb[:, 512 * t:512 * t + 512],
                        op0=ALU.mult, op1=ALU.add)
                    nc.sync.dma_start(
                        out=out[128 * f:128 * f + 128, 512 * t:512 * t + 512],
                        in_=ob[:])

    nc.compile()
    return nc


def shard_inputs(inp, W, conv_w, conv_b, sk_wt, fin=FIN):
    """Build the 8 per-core input maps (host: layout/constants only)."""
    tok = inp.size // fin
    f8 = mybir.dt.np(FP8)
    bf = mybir.dt.np(BF16)
    n_k = fin // 128

    inp2 = np.asarray(inp, dtype=np.float32).reshape(tok, fin)
    W = np.asarray(W, dtype=np.float32)
    cw = np.asarray(conv_w, dtype=np.float32).reshape(NUM_HEADS, 3, 3)
    cb = np.asarray(conv_b, dtype=np.float32).reshape(NUM_HEADS)
    sk = np.asarray(sk_wt, dtype=np.float32).reshape(NUM_HEADS)
    sig = 1.0 / (1.0 + np.exp(-sk))

    # x^T in fp8 (replicated), rank-1 rowsum term in f32
    xT8 = np.ascontiguousarray(inp2.T).astype(f8)
    rs = inp2.astype(np.float64).sum(axis=1).astype(np.float32)  # [tok]

    hsz = W.shape[0] // NUM_HEADS
    in_maps = []
    for c in range(NCORES):
        gr0 = c * FSH
        h = (gr0 // hsz) % NUM_HEADS
        lo = max(gr0 - 1, h * hsz)          # fout halo, head-clamped
        hi = min(gr0 + FSH + 1, (h + 1) * hsz)
        # W^T shard with fout-halo: wth[c_, j] = W[gr0 + j - 1, c_]
        wthal = np.zeros((fin, FSH + 2), dtype=np.float32)
        wthal[:, lo - (gr0 - 1):hi - (gr0 - 1)] = W[lo:hi, :fin].T
        # conv halo rows: hh[k, 3r+dr, j] = W[gr0+j+dr-1, c_r], head-clamped
        hhv = np.zeros((n_k, 6, FSH), dtype=np.float32)
        for k in range(n_k):
            for r, c_r in ((0, 128 * k - 1), (1, 128 * k + 128)):
                if c_r < 0 or c_r >= fin:
                    continue
                col = W[lo:hi, c_r]
                for dr in range(3):
                    j0 = lo - (gr0 + dr - 1)
                    jlo = max(0, j0)
                    jhi = min(FSH, hi - (gr0 + dr - 1))
                    if jhi > jlo:
                        hhv[k, 3 * r + dr, jlo:jhi] = col[jlo - j0:jhi - j0]
        # banded matrices B_dr[c_in, c_out] = cw[dr, dc] where
        # c_in - c_out == dc - 1; sigmoid residual on B_1's main diagonal
        b3v = np.zeros((128, 3, 128), dtype=np.float32)
        idx = np.arange(127)
        for dr in range(3):
            b3v[idx, dr, idx + 1] = cw[h, dr, 0]
            b3v[np.arange(128), dr, np.arange(128)] = cw[h, dr, 1]
            b3v[idx + 1, dr, idx] = cw[h, dr, 2]
        b3v[np.arange(128), 1, np.arange(128)] += sig[h]
        # halo coefficients: row (r=0,dr) -> c_out 0 tap cw[dr,0];
        #                    row (r=1,dr) -> c_out 127 tap cw[dr,2]
        hmv = np.zeros((6, 128), dtype=np.float32)
        for dr in range(3):
            hmv[dr, 0] = cw[h, dr, 0]
            hmv[3 + dr, 127] = cw[h, dr, 2]
        rsbv = np.ascontiguousarray(
            np.broadcast_to(cb[h] * rs, (128, tok))).astype(np.float32)
        in_maps.append({
            "xs": xT8,
            "wth": wthal.astype(bf),
            "hh": hhv.astype(bf),
            "b3": b3v.astype(bf),
            "hm": hmv.astype(bf),
            "rsb": rsbv,
        })
    return in_maps


_PROGRAM_CACHE = {}


def _get_program(tok, fin):
    key = (tok, fin)
    if key not in _PROGRAM_CACHE:
        _PROGRAM_CACHE[key] = build_program(tok, fin)
    return _PROGRAM_CACHE[key]


def kernel(inp, W, conv_w, conv_b, sk_wt):
    nc = _get_program(TOK, FIN)
    in_maps = shard_inputs(inp, W, conv_w, conv_b, sk_wt)
    res = run_bass_kernel_spmd(nc, in_maps, list(range(NCORES)))
    # out^T shards [FSH, tok] -> [fout, tok] -> transpose to [tok, fout]
    oT = np.concatenate([res.results[c]["o"] for c in range(NCORES)], axis=0)
    return np.ascontiguousarray(oT.T.reshape(2, TOK // 2, FOUT)
                                ).astype(np.float32)
